# revision 1
# baseline (speedup 1.0000x reference)
"""Trainium2 Bass kernel for nn_Decoder (LSTM decoder with residual output feedback).

Model (per batch row):
    h0 = c0 = z @ W_proj.T + b_proj                      # [B, H]
    y0 = x[:, -1, :]                                     # [B, X]
    per step t: gates = y_{t-1} @ W_ih.T + h_{t-1} @ W_hh.T + (b_ih + b_hh)
                i, f, g, o = split(gates); c = sig(f)*c + sig(i)*tanh(g)
                h = sig(o)*tanh(c); y_t = y_{t-1} + h @ W_out.T + b_out
    out = stack(y_1..y_T)                                # [B, T, Y]

Strategy:
  * Pure data-parallel over batch: B=1024 -> 128 rows/core on 8 NeuronCores,
    weights replicated, zero collectives; outputs concatenated on the host.
  * All state is kept TRANSPOSED on chip ([feature, batch], batch on the free
    axis): gates come out of the PE array as gates^T with the weights as the
    stationary operand, and the elementwise state update directly produces
    h^T, which feeds the next step's matmuls as the moving operand -- the
    recurrence contains no transposes at all.
  * Each core's 128 rows are further split into two independent 64-row halves
    executed half-a-step out of phase: while one half runs its sigmod/tanh +
    c/h update chain on the Scalar/Vector engines, the other half's matmul
    burst keeps the TensorEngine busy (and the PE clock un-throttled).
  * Matmuls run in bf16 (f32 PSUM accumulation); c and y state stay f32.
  * Gate rows are pre-permuted on the host so PSUM holds [i_j|f_j|o_j|g_j]
    blocks, making the activations strided-AP single instructions.
  * y_t accumulates in a persistent PSUM bank (y_t = y0 + psum); the output
    is staged transposed in SBUF and un-transposed on the host.
"""



import os
from contextlib import ExitStack

import ml_dtypes
import numpy as np

import concourse.bass as bass
import concourse.tile as tile
from concourse import bacc, mybir
from concourse.bass_utils import run_bass_kernel_spmd


F32 = mybir.dt.float32
BF16 = mybir.dt.bfloat16
SIG = mybir.ActivationFunctionType.Sigmoid
TANH = mybir.ActivationFunctionType.Tanh

B_TOT = 1024
N_CORES = 8
B = 128          # rows per core
BH = 64          # rows per half
ZD, XD, YD, H = 128, 64, 64, 512
HC, GC = 4, 16

LAST_RESULTS = None
_BUILD_CACHE = {}



def _gate_row_order():
    """Permuted gate-row order: for H-chunk j, the 128-row blocks (i_j, f_j, o_j, g_j).

    Original gate layout along 4H: i=[0,512), f=[512,1024), g=[1024,1536), o=[1536,2048).
    """
    idx = []
    for j in range(HC):
        for base in (0, 512, 1536, 1024):  # i, f, o, g
            idx.extend(range(base + j * 128, base + (j + 1) * 128))
    return np.asarray(idx)



def _prep_consts(W_ih, W_hh, b_ih, b_hh, W_proj, b_proj, W_out, b_out):
    bf = ml_dtypes.bfloat16
    order = _gate_row_order()
    Wg = W_hh[order]                       # [2048, 512] permuted rows
    Wi = W_ih[order]                       # [2048, 64]
    bt = (b_ih + b_hh)[order]              # [2048]

    wg_h = np.empty((128, HC * GC * 128), dtype=bf)
    for k in range(HC):
        for s in range(GC):
            blk = Wg[s * 128:(s + 1) * 128, k * 128:(k + 1) * 128].T  # [K,M]
            wg_h[:, (k * GC + s) * 128:(k * GC + s + 1) * 128] = blk.astype(bf)

    wg_y = np.empty((YD + 1, GC * 128), dtype=bf)
    for s in range(GC):
        wg_y[0:YD, s * 128:(s + 1) * 128] = Wi[s * 128:(s + 1) * 128, :].T.astype(bf)
        wg_y[YD, s * 128:(s + 1) * 128] = bt[s * 128:(s + 1) * 128].astype(bf)

    wout = np.zeros((128, HC * 128), dtype=bf)
    for k in range(HC):
        wout[:, k * 128:k * 128 + YD] = W_out[:, k * 128:(k + 1) * 128].T.astype(bf)

    wproj = np.empty((ZD, H), dtype=bf)
    for m in range(HC):
        wproj[:, m * 128:(m + 1) * 128] = W_proj[m * 128:(m + 1) * 128, :].T.astype(bf)

    bprojT = b_proj.reshape(HC, 128).T.copy().astype(np.float32)  # [128, HC]
    bout1 = b_out.reshape(1, YD).astype(bf)
    ones1 = np.ones((1, B), dtype=bf)
    return dict(wg_h=wg_h, wg_y=wg_y, wout=wout, wproj=wproj,
                bprojT=bprojT, bout1=bout1, ones1=ones1)



def _j4(ap, c):
    return ap.rearrange("p (j c) -> p j c", c=c)


def _build(T):
    nc = bacc.Bacc("TRN2", target_bir_lowering=False, debug=False)

    d_zT = nc.dram_tensor("zT", [ZD, B], BF16, kind="ExternalInput")
    d_y0T = nc.dram_tensor("y0T", [YD, B], F32, kind="ExternalInput")
    d_wg_h = nc.dram_tensor("wg_h", [128, HC * GC * 128], BF16, kind="ExternalInput")
    d_wg_y = nc.dram_tensor("wg_y", [YD + 1, GC * 128], BF16, kind="ExternalInput")
    d_wout = nc.dram_tensor("wout", [128, HC * 128], BF16, kind="ExternalInput")
    d_bout1 = nc.dram_tensor("bout1", [1, YD], BF16, kind="ExternalInput")
    d_ones1 = nc.dram_tensor("ones1", [1, B], BF16, kind="ExternalInput")
    d_wproj = nc.dram_tensor("wproj", [ZD, H], BF16, kind="ExternalInput")
    d_bprojT = nc.dram_tensor("bprojT", [128, HC], F32, kind="ExternalInput")
    d_bscan = nc.dram_tensor("bscan", [YD, T], F32, kind="ExternalInput")
    d_out = nc.dram_tensor("out", [YD, T * B], F32, kind="ExternalOutput")

    with ExitStack() as ctx:
        tc = ctx.enter_context(tile.TileContext(nc))
        const = ctx.enter_context(tc.tile_pool(name="const", bufs=1))
        state = ctx.enter_context(tc.tile_pool(name="state", bufs=1))
        actp = ctx.enter_context(tc.tile_pool(name="actp", bufs=4))
        gpsum = ctx.enter_context(tc.tile_pool(name="gpsum", bufs=3, space="PSUM"))
        ypsum = ctx.enter_context(tc.tile_pool(name="ypsum", bufs=1, space="PSUM"))

        wg_h = const.tile([128, HC * GC * 128], BF16)
        wg_y = const.tile([YD + 1, GC * 128], BF16)
        wout = const.tile([128, HC * 128], BF16)
        bout1 = const.tile([1, YD], BF16)
        ones1 = const.tile([1, B], BF16)
        wproj = const.tile([ZD, H], BF16)
        bprojT = const.tile([128, HC], F32)
        bscan = const.tile([YD, T], F32)
        zT = const.tile([ZD, B], BF16)
        y0T = const.tile([YD, B], F32)
        for sb, dr in (
            (wg_h, d_wg_h), (wg_y, d_wg_y), (wout, d_wout), (bout1, d_bout1),
            (ones1, d_ones1), (wproj, d_wproj), (bprojT, d_bprojT),
            (bscan, d_bscan),
            (zT, d_zT), (y0T, d_y0T),
        ):
            nc.sync.dma_start(sb[:, :], dr[:, :])

        # per-half state; layout [128, 4*64]: H-chunk k at cols k*64
        cT = [state.tile([128, 256], F32, name=f"cT{h}") for h in range(2)]
        hT = [[state.tile([128, 256], BF16, name=f"hT{p}_{h}") for h in range(2)]
              for p in range(2)]
        yTa = [[state.tile([YD + 1, BH], BF16, name=f"yTa{p}_{h}") for h in range(2)]
               for p in range(2)]
        ysbT = state.tile([YD, T * B], F32)   # out[y, t*128 + h*64 + b]
        yp = [ypsum.tile([128, BH], F32, name=f"yp{h}", tag=f"yp{h}")
              for h in range(2)]

        # --- init (both halves) ---
        for h in range(2):
            bsl = slice(h * BH, (h + 1) * BH)
            h0p = gpsum.tile([128, 1024], F32, tag="g", name=f"h0p{h}")
            for m in range(HC):
                nc.tensor.matmul(
                    h0p[:, m * 64:(m + 1) * 64],
                    lhsT=wproj[:, m * 128:(m + 1) * 128],
                    rhs=zT[:, bsl], start=True, stop=True,
                )
            for m in range(HC):
                nc.vector.tensor_scalar_add(
                    cT[h][:, m * 64:(m + 1) * 64],
                    h0p[:, m * 64:(m + 1) * 64],
                    bprojT[:, m:m + 1],
                )
            nc.vector.tensor_copy(hT[1][h][:, :], cT[h][:, :])
            nc.vector.tensor_copy(yTa[1][h][0:YD, :], y0T[:, bsl])
            nc.vector.memset(yTa[0][h][YD:YD + 1, :], 1.0)
            nc.vector.memset(yTa[1][h][YD:YD + 1, :], 1.0)

        DMA_CHUNK = 32

        def emit_half(t, h):
            pv = (t + 1) % 2
            cu = t % 2
            bsl = slice(h * BH, (h + 1) * BH)
            gp = gpsum.tile([128, 1024], F32, tag="g", name=f"gp{t}_{h}")

            for k in range(HC):
                if t > 0:
                    nc.tensor.matmul(
                        yp[h][:, :],
                        lhsT=wout[:, k * 128:(k + 1) * 128],
                        rhs=hT[pv][h][:, k * 64:(k + 1) * 64],
                        start=(t == 1 and k == 0), stop=False,
                        skip_group_check=True,
                    )
                for s in range(GC):
                    nc.tensor.matmul(
                        gp[:, s * 64:(s + 1) * 64],
                        lhsT=wg_h[:, (k * GC + s) * 128:(k * GC + s + 1) * 128],
                        rhs=hT[pv][h][:, k * 64:(k + 1) * 64],
                        start=(k == 0 and s % 8 == 0), stop=False,
                        skip_group_check=True,
                    )
            if t > 0:
                tp = t - 1
                # y_tp = y0 + psum + (tp+1)*b_out; the bias ramp comes from a
                # host-precomputed table so no bias matmul sits on this path
                nc.vector.scalar_tensor_tensor(
                    yTa[tp % 2][h][0:YD, :], yp[h][0:YD, :], bscan[:, tp:tp + 1],
                    y0T[:, bsl], op0=mybir.AluOpType.add, op1=mybir.AluOpType.add)
                sl = ysbT[:, tp * B + h * BH:tp * B + (h + 1) * BH]
                nc.vector.scalar_tensor_tensor(
                    sl, yp[h][0:YD, :], bscan[:, tp:tp + 1],
                    y0T[:, bsl], op0=mybir.AluOpType.add, op1=mybir.AluOpType.add)
                if h == 1 and (tp % DMA_CHUNK == DMA_CHUNK - 1):
                    lo = (tp // DMA_CHUNK) * DMA_CHUNK * B
                    nc.sync.dma_start(d_out[:, lo:(tp + 1) * B],
                                      ysbT[:, lo:(tp + 1) * B])
            for s in range(GC):
                nc.tensor.matmul(
                    gp[:, s * 64:(s + 1) * 64],
                    lhsT=wg_y[:, s * 128:(s + 1) * 128],
                    rhs=yTa[pv][h][:, :],
                    start=False, stop=True, skip_group_check=True,
                )

            gp4 = _j4(gp, 256)
            sg = actp.tile([128, 768], F32, tag=f"sg{h}", name=f"sg{t}_{h}")
            tg = actp.tile([128, 256], F32, tag=f"tg{h}", name=f"tg{t}_{h}")
            sg4 = _j4(sg, 192)
            tg4 = _j4(tg, 64)
            # sigma(i,f) first so the c-update chain starts as early as possible
            nc.scalar.activation(sg4[:, :, 0:128], gp4[:, :, 0:128], SIG)
            nc.scalar.activation(tg4, gp4[:, :, 192:256], TANH)
            cs4 = _j4(cT[h], 64)
            t2 = actp.tile([128, 256], F32, tag=f"t2{h}", name=f"t2_{t}_{h}")
            nc.vector.tensor_mul(_j4(t2, 64), sg4[:, :, 64:128], cs4)
            t1 = actp.tile([128, 256], F32, tag=f"t1{h}", name=f"t1_{t}_{h}")
            nc.vector.tensor_mul(_j4(t1, 64), sg4[:, :, 0:64], tg4)
            nc.vector.tensor_add(cs4, _j4(t2, 64), _j4(t1, 64))
            nc.scalar.activation(sg4[:, :, 128:192], gp4[:, :, 128:192], SIG)
            tch = actp.tile([128, 256], F32, tag=f"tc{h}", name=f"tc{t}_{h}")
            nc.scalar.activation(_j4(tch, 64), cs4, TANH)
            nc.vector.tensor_mul(_j4(hT[cu][h], 64), sg4[:, :, 128:192],
                                 _j4(tch, 64))

        for t in range(T):
            emit_half(t, 0)
            emit_half(t, 1)

        # final y tails
        for h in range(2):
            bsl = slice(h * BH, (h + 1) * BH)
            for k in range(HC):
                nc.tensor.matmul(
                    yp[h][:, :],
                    lhsT=wout[:, k * 128:(k + 1) * 128],
                    rhs=hT[(T - 1) % 2][h][:, k * 64:(k + 1) * 64],
                    start=False, stop=(k == HC - 1), skip_group_check=True,
                )
            tp = T - 1
            sl = ysbT[:, tp * B + h * BH:tp * B + (h + 1) * BH]
            nc.vector.scalar_tensor_tensor(
                sl, yp[h][0:YD, :], bscan[:, tp:tp + 1],
                y0T[:, bsl], op0=mybir.AluOpType.add, op1=mybir.AluOpType.add)
        lo = ((T - 1) // DMA_CHUNK) * DMA_CHUNK * B
        nc.sync.dma_start(d_out[:, lo:T * B], ysbT[:, lo:T * B])

    nc.compile()
    return nc


def kernel(z, x, W_ih, W_hh, b_ih, b_hh, W_proj, b_proj, W_out, b_out, y_pred_len):
    global LAST_RESULTS
    z = np.asarray(z, dtype=np.float32)
    x = np.asarray(x, dtype=np.float32)
    T = int(np.asarray(y_pred_len))

    consts = _prep_consts(
        np.asarray(W_ih, np.float32), np.asarray(W_hh, np.float32),
        np.asarray(b_ih, np.float32), np.asarray(b_hh, np.float32),
        np.asarray(W_proj, np.float32), np.asarray(b_proj, np.float32),
        np.asarray(W_out, np.float32), np.asarray(b_out, np.float32),
    )

    if T not in _BUILD_CACHE:
        _BUILD_CACHE[T] = _build(T)
    nc = _BUILD_CACHE[T]
    consts["bscan"] = np.ascontiguousarray(
        np.outer(np.asarray(b_out, np.float32),
                 np.arange(1, T + 1, dtype=np.float32)))

    bf = ml_dtypes.bfloat16
    in_maps = []
    for i in range(N_CORES):
        sl = slice(i * B, (i + 1) * B)
        m = dict(consts)
        m["zT"] = np.ascontiguousarray(z[sl].T.astype(bf))
        m["y0T"] = np.ascontiguousarray(x[sl, -1, :].T.astype(np.float32))
        in_maps.append(m)

    trace = bool(int(os.environ.get("BASS_KERNEL_TRACE", "0")))
    res = run_bass_kernel_spmd(
        nc, in_maps, core_ids=list(range(N_CORES)), trace=trace,
    )
    LAST_RESULTS = res

    outs = [np.ascontiguousarray(
                np.asarray(res.results[i]["out"]).reshape(YD, T, B).transpose(2, 1, 0))
            for i in range(N_CORES)]
    return np.concatenate(outs, axis=0)

